# revision 1
# baseline (speedup 1.0000x reference)
"""DCRNN decoder (2-layer DCGRU, diffusion graph conv) on 8 trn2 cores.

Sharding: data-parallel over batch B=64 -> 8 batches/core; supports and
weights replicated. No collectives.

Per-core dataflow (all SBUF-resident after initial load):
  state kept in two layouts:
    natural  [node_part, feat]  -> stationary (lhsT) for aggregation matmuls
    transposed [feat_part, node] -> elementwise gate math + PE-transposes
  aggregation: aggrT[f, n_out] = sum_k feat[k, f] * S_T[m][k, n_out]
     (bf16 x bf16 -> fp32 PSUM, accumulated over 8 k-tiles)
  projection:  gate[gh, n_out] += W[m][f, gh]^T @ aggrT (fp32r, PSUM over m)
  gates: ACT sigmoid/tanh (+bias) -> bf16 transposed tiles
  update: h' = c + u*(h-c) on DVE; PE-transpose pairs of batches back to
  natural layout for the next aggregation.
"""

import sys

import numpy as np
import ml_dtypes

for _p in ("/opt/trn_rl_repo", "/root/.axon_site/_ro/trn_rl_repo"):
    if _p not in sys.path:
        sys.path.append(_p)

import concourse.bass as bass
import concourse.mybir as mybir
import concourse.tile as tile
from concourse.bass import ds
from concourse.bass_utils import run_bass_kernel_spmd

F32 = mybir.dt.float32
F32R = mybir.dt.float32r
BF16 = mybir.dt.bfloat16

NCORES = 8
BC = 8          # batches per core
N = 1000        # nodes
NPAD = 1024
KT = 8          # node (contraction) tiles of 128
NT = 8          # node output tiles of 128
H = 64
M = 4           # supports
NSTEP = 11      # time steps (T-1)
CH = 500        # n_out chunk (2 chunks of 500 per matmul free dim)
NPAIR = 4       # batch pairs

bf16 = ml_dtypes.bfloat16


def _nt_slice(nt):
    lo = 128 * nt
    hi = min(lo + 128, N)
    return lo, hi


def build_program(use_f32r_proj=True, dyn_loop=True, nstep=NSTEP, state_f32=True):
    nc = bass.Bass()
    SDT = F32 if state_f32 else BF16

    # ---- DRAM I/O ----
    st_d = nc.dram_tensor("st", [128, M, KT, N], BF16, kind="ExternalInput")
    h0n_d = nc.dram_tensor("h0n", [128, KT, BC, H], BF16, kind="ExternalInput")
    h1n_d = nc.dram_tensor("h1n", [128, KT, BC, H], BF16, kind="ExternalInput")
    h0t_d = nc.dram_tensor("h0t", [128, NPAIR, N], SDT, kind="ExternalInput")
    h1t_d = nc.dram_tensor("h1t", [128, NPAIR, N], SDT, kind="ExternalInput")
    xseq_d = nc.dram_tensor("xseq", [nstep, 128, KT, BC], BF16, kind="ExternalInput")
    w0ru_d = nc.dram_tensor("w0ru", [128, M, 128], F32R, kind="ExternalInput")
    w0c_d = nc.dram_tensor("w0c", [128, M, H], F32R, kind="ExternalInput")
    w1ru_d = nc.dram_tensor("w1ru", [128, M, 128], F32R, kind="ExternalInput")
    w1c_d = nc.dram_tensor("w1c", [128, M, H], F32R, kind="ExternalInput")
    bias_d = nc.dram_tensor("biases", [128, 6], F32, kind="ExternalInput")
    wproj_d = nc.dram_tensor("wproj", [128, 2], SDT, kind="ExternalInput")
    pbias_d = nc.dram_tensor("pbias", [2, 1], F32, kind="ExternalInput")
    identf_d = nc.dram_tensor("identf", [128, 128], SDT, kind="ExternalInput")
    identb_d = nc.dram_tensor("identb", [128, 128], BF16, kind="ExternalInput")
    out_d = nc.dram_tensor("out", [BC, nstep, N], F32, kind="ExternalOutput")

    pdt = F32R if use_f32r_proj else F32

    with tile.TileContext(nc) as tc:
        with (
            tc.tile_pool(name="const", bufs=1) as const,
            tc.tile_pool(name="agsb", bufs=4) as agsb_pool,
            tc.tile_pool(name="gates", bufs=4) as gate_pool,
            tc.tile_pool(name="upool", bufs=3) as u_pool,
            tc.tile_pool(name="hload", bufs=2) as hload_pool,
            tc.tile_pool(name="htmp", bufs=2) as htmp_pool,
            tc.tile_pool(name="outb", bufs=1) as out_pool,
            tc.tile_pool(name="ps_agg", bufs=3, space="PSUM") as ps_agg,
            tc.tile_pool(name="ps_gate", bufs=3, space="PSUM") as ps_gate,
            tc.tile_pool(name="ps_tp", bufs=2, space="PSUM") as ps_tp,
        ):
            # ---- resident tiles ----
            st = const.tile([128, M, KT, N], BF16, tag="st")
            feat0 = const.tile([128, KT, BC, 65], BF16, tag="feat0")
            feat1 = const.tile([128, KT, BC, 128], BF16, tag="feat1")
            h0t = const.tile([128, NPAIR, N], SDT, tag="h0t")
            h1t = const.tile([128, NPAIR, N], SDT, tag="h1t")
            rhT = const.tile([128, NPAIR, N], BF16, tag="rhT")
            w0ru = const.tile([128, M, 128], F32R, tag="w0ru")
            w0c = const.tile([128, M, H], F32R, tag="w0c")
            w1ru = const.tile([128, M, 128], F32R, tag="w1ru")
            w1c = const.tile([128, M, H], F32R, tag="w1c")
            biases = const.tile([128, 6], F32, tag="biases")
            wproj = const.tile([128, 2], SDT, tag="wproj")
            pbias = const.tile([2, 1], F32, tag="pbias")
            identf = const.tile([128, 128], SDT, tag="identf")
            identb = const.tile([128, 128], BF16, tag="identb")

            # ---- initial loads ----
            nc.vector.memset(feat0, 0.0)
            nc.vector.memset(feat1, 0.0)
            nc.sync.dma_start(out=st, in_=st_d[:])
            nc.sync.dma_start(out=feat0[:, :, :, 1:65], in_=h0n_d[:])
            nc.sync.dma_start(out=feat1[:, :, :, 64:128], in_=h1n_d[:])
            nc.sync.dma_start(out=h0t, in_=h0t_d[:])
            nc.sync.dma_start(out=h1t, in_=h1t_d[:])
            nc.sync.dma_start(out=w0ru, in_=w0ru_d[:])
            nc.sync.dma_start(out=w0c, in_=w0c_d[:])
            nc.sync.dma_start(out=w1ru, in_=w1ru_d[:])
            nc.sync.dma_start(out=w1c, in_=w1c_d[:])
            nc.sync.dma_start(out=biases, in_=bias_d[:])
            nc.sync.dma_start(out=wproj, in_=wproj_d[:])
            nc.sync.dma_start(out=pbias, in_=pbias_d[:])
            nc.sync.dma_start(out=identf, in_=identf_d[:])
            nc.sync.dma_start(out=identb, in_=identb_d[:])

            def aggr_block(b, feat, fwid, ch):
                """Aggregation for one (batch, chunk): 4 per-m aggrT tiles."""
                js = ds(ch * CH, CH)
                ags = []
                for m in range(M):
                    ag_ps = ps_agg.tile([128, CH], F32, tag="agg")
                    for kt in range(KT):
                        nc.tensor.matmul(
                            ag_ps[0:fwid, :],
                            lhsT=feat[:, kt, b, 0:fwid],
                            rhs=st[:, m, kt, js],
                            start=(kt == 0),
                            stop=(kt == KT - 1),
                        )
                    ag = agsb_pool.tile([128, CH], F32R, tag="agsb")
                    nc.vector.tensor_copy(ag[0:fwid, :], ag_ps[0:fwid, :])
                    ags.append(ag)
                return ags

            def proj_gate(ags, fwid, wtile, wcol, bcol, act_fn, out_sb, ch):
                """out_sb[0:64, ch] = act(sum_m W[m][:,wcol:wcol+64]^T @ aggrT + b)."""
                js = ds(ch * CH, CH)
                g_ps = ps_gate.tile([H, CH], F32, tag="gate")
                for m in range(M):
                    nc.tensor.matmul(
                        g_ps,
                        lhsT=wtile[0:fwid, m, wcol : wcol + H],
                        rhs=ags[m][0:fwid, :],
                        start=(m == 0),
                        stop=(m == M - 1),
                    )
                nc.scalar.activation(
                    out=out_sb[0:H, js],
                    in_=g_ps,
                    func=act_fn,
                    bias=biases[0:H, bcol : bcol + 1],
                )

            def transpose_to(src_tile, p, dests):
                """PE-transpose src_tile[:, p, :] (bf16 [128, N]) into natural
                layout; write [node, 2b*64f] to each dest (feat, col_lo)."""
                sdt = src_tile.dtype
                idt = identb if sdt == BF16 else identf
                for nt in range(NT):
                    lo, hi = _nt_slice(nt)
                    w = hi - lo
                    tp = ps_tp.tile([128, 128], sdt, tag="tp")
                    nc.tensor.transpose(
                        tp[0:w, :], src_tile[:, p, lo:hi], idt
                    )
                    for feat, col in dests:
                        nc.vector.tensor_copy(
                            feat[0:w, nt, 2 * p : 2 * p + 2, col : col + H],
                            tp[0:w, :].rearrange("p (b f) -> p b f", b=2),
                        )

            def layer(lidx, t_iv):
                feat = feat0 if lidx == 0 else feat1
                fwid = 65 if lidx == 0 else 128
                wru = w0ru if lidx == 0 else w1ru
                wc = w0c if lidx == 0 else w1c
                ht = h0t if lidx == 0 else h1t
                bcol = 3 * lidx
                rh_col = 1 if lidx == 0 else 64
                SIG = mybir.ActivationFunctionType.Sigmoid
                TANH = mybir.ActivationFunctionType.Tanh

                for p in range(NPAIR):
                    uT = {}
                    hsl = {}
                    for b in (2 * p, 2 * p + 1):
                        odd = b % 2
                        if odd:
                            hsrc = hload_pool.tile([H, N], SDT, tag="hload")
                            nc.sync.dma_start(out=hsrc, in_=ht[H:128, p, :])
                            hsl[b] = hsrc
                        else:
                            hsl[b] = ht[0:H, p, :]
                        rT = gate_pool.tile([H, N], SDT, tag="rT")
                        u_tile = u_pool.tile([H, N], SDT, tag="uT")
                        uT[b] = u_tile
                        for ch in range(2):
                            ags = aggr_block(b, feat, fwid, ch)
                            proj_gate(ags, fwid, wru, 0, bcol, SIG, rT, ch)
                            proj_gate(ags, fwid, wru, H, bcol + 1, SIG, u_tile, ch)
                        # rh = r * h  (transposed layout, base 0)
                        if odd:
                            rh_tmp = gate_pool.tile([H, N], BF16, tag="rT")
                            nc.vector.tensor_mul(rh_tmp, rT, hsl[b])
                            nc.sync.dma_start(out=rhT[H:128, p, :], in_=rh_tmp)
                        else:
                            nc.vector.tensor_mul(rhT[0:H, p, :], rT, hsl[b])
                    # rh -> natural (overwrites h cols of feat; h cols dead)
                    transpose_to(rhT, p, [(feat, rh_col)])
                    for b in (2 * p, 2 * p + 1):
                        odd = b % 2
                        cT = gate_pool.tile([H, N], SDT, tag="cT")
                        for ch in range(2):
                            ags = aggr_block(b, feat, fwid, ch)
                            proj_gate(ags, fwid, wc, 0, bcol + 2, TANH, cT, ch)
                        tmp = htmp_pool.tile([H, N], SDT, tag="htmp")
                        # h' = c + u*(h-c)
                        nc.vector.tensor_sub(tmp, hsl[b], cT)
                        nc.vector.tensor_mul(tmp, uT[b], tmp)
                        if odd:
                            hnew = htmp_pool.tile([H, N], SDT, tag="htmp")
                            nc.vector.tensor_add(hnew, cT, tmp)
                            nc.sync.dma_start(out=ht[H:128, p, :], in_=hnew)
                        else:
                            nc.vector.tensor_add(ht[0:H, p, :], cT, tmp)
                    if lidx == 0:
                        # h0' natural -> feat1 x-part and feat0 state cols
                        transpose_to(h0t, p, [(feat1, 0), (feat0, 1)])
                    else:
                        # h1' natural -> feat1 state cols
                        transpose_to(h1t, p, [(feat1, 64)])
                        # output projection for this pair
                        ob = out_pool.tile([2, N], F32, tag="outb")
                        for ch in range(2):
                            js = ds(ch * CH, CH)
                            o_ps = ps_tp.tile([2, CH], F32, tag="tp")
                            nc.tensor.matmul(
                                o_ps,
                                lhsT=wproj[:],
                                rhs=h1t[:, p, js],
                                start=True,
                                stop=True,
                            )
                            nc.scalar.activation(
                                out=ob[:, js],
                                in_=o_ps,
                                func=mybir.ActivationFunctionType.Identity,
                                bias=pbias[:],
                            )
                        nc.sync.dma_start(
                            out=out_d[2 * p : 2 * p + 2, ds(t_iv, 1), :].squeeze(1),
                            in_=ob,
                        )

            def step_body(t_iv):
                nc.sync.dma_start(
                    out=feat0[:, :, :, 0:1].squeeze(3),
                    in_=xseq_d[ds(t_iv, 1), :, :, :].squeeze(0),
                )
                layer(0, t_iv)
                layer(1, t_iv)

            if dyn_loop:
                with tc.For_i(0, nstep, 1, hint_engines=(mybir.EngineType.PE,)) as t:
                    step_body(t)
            else:
                for t in range(nstep):
                    step_body(t)

    _split_excess_waits(nc)
    return nc


def prep_inputs(inputs, state_f32=True):
    sdt = np.float32 if state_f32 else bf16
    """Host-side shard + relayout. Returns list of per-core in_maps."""
    S = np.asarray(inputs["supports"], np.float32)          # [M,N,N]
    ih = np.asarray(inputs["init_hidden"], np.float32)      # [2,B,N,H]
    x = np.asarray(inputs["input"], np.float32)[:, :, :, 0]  # [B,T,N]
    B = x.shape[0]

    # supports, transposed + padded: st[p,m,kt,j] = S[m][j,128kt+p]
    Sp = np.zeros((M, N, NPAD), np.float32)
    Sp[:, :, :N] = S
    st = Sp.reshape(M, N, KT, 128).transpose(3, 0, 2, 1).astype(bf16).copy()

    # weights
    f0 = 1 + H
    w0ru = np.zeros((128, M, 128), np.float32)
    w0c = np.zeros((128, M, H), np.float32)
    w1ru = np.zeros((128, M, 128), np.float32)
    w1c = np.zeros((128, M, H), np.float32)
    for m in range(M):
        w0ru[0:f0, m, 0:H] = inputs["w0_r"][m]
        w0ru[0:f0, m, H:128] = inputs["w0_u"][m]
        w0c[0:f0, m, :] = inputs["w0_c"][m]
        w1ru[:, m, 0:H] = inputs["w1_r"][m]
        w1ru[:, m, H:128] = inputs["w1_u"][m]
        w1c[:, m, :] = inputs["w1_c"][m]
    biases = np.zeros((128, 6), np.float32)
    for half in (0, 1):
        r0, r1 = half * H, half * H + H
        biases[r0:r1, 0] = inputs["b0_r"]
        biases[r0:r1, 1] = inputs["b0_u"]
        biases[r0:r1, 2] = inputs["b0_c"]
        biases[r0:r1, 3] = inputs["b1_r"]
        biases[r0:r1, 4] = inputs["b1_u"]
        biases[r0:r1, 5] = inputs["b1_c"]
    wproj = np.zeros((128, 2), np.float32)
    wproj[0:H, 0] = np.asarray(inputs["proj_w"], np.float32)[:, 0]
    wproj[H:128, 1] = np.asarray(inputs["proj_w"], np.float32)[:, 0]
    wproj = wproj.astype(sdt)
    pbias = np.full((2, 1), np.asarray(inputs["proj_b"], np.float32).reshape(()),
                    np.float32)
    identf = np.eye(128, dtype=sdt)
    identb = np.eye(128, dtype=bf16)

    common = dict(st=st, w0ru=w0ru, w0c=w0c, w1ru=w1ru, w1c=w1c,
                  biases=biases, wproj=wproj, pbias=pbias, identf=identf, identb=identb)

    in_maps = []
    for core in range(NCORES):
        bsl = slice(core * BC, (core + 1) * BC)
        ihc = ih[:, bsl]                                    # [2,8,N,H]
        ihp = np.zeros((2, BC, NPAD, H), np.float32)
        ihp[:, :, :N] = ihc
        hn = ihp.reshape(2, BC, KT, 128, H).transpose(0, 3, 2, 1, 4)  # [2,128,KT,BC,H]
        htr = ihc.transpose(0, 1, 3, 2).reshape(2, NPAIR, 2, H, N)
        htr = htr.transpose(0, 1, 2, 3, 4).reshape(2, NPAIR, 2 * H, N)
        htr = htr.transpose(0, 2, 1, 3)                     # [2,128,NPAIR,N]
        xc = x[bsl, :NSTEP]                                 # [8,11,N]
        xp = np.zeros((BC, NSTEP, NPAD), np.float32)
        xp[:, :, :N] = xc
        xseq = xp.reshape(BC, NSTEP, KT, 128).transpose(1, 3, 2, 0)  # [11,128,KT,BC]
        in_maps.append(dict(
            common,
            h0n=hn[0].astype(bf16).copy(),
            h1n=hn[1].astype(bf16).copy(),
            h0t=htr[0].astype(sdt).copy(),
            h1t=htr[1].astype(sdt).copy(),
            xseq=xseq.astype(bf16).copy(),
        ))
    return in_maps



_WAIT_CAP = 1


def _split_excess_waits(nc):
    """Walrus codegen here accepts at most 2 sync-wait commands per
    instruction; Tile can emit more.  Move excess waits onto injected
    same-engine no-ops placed immediately before the instruction."""
    for fn in nc.m.functions:
        for blk in fn.blocks:
            insts = list(blk.instructions)
            out = []
            for inst in insts:
                si = getattr(inst, "sync_info", None)
                waits = list(si.on_wait) if si and si.on_wait else []
                if len(waits) > _WAIT_CAP:
                    extra, keep = waits[:-_WAIT_CAP], waits[-_WAIT_CAP:]
                    while extra:
                        chunk, extra = extra[:_WAIT_CAP], extra[_WAIT_CAP:]
                        out.append(mybir.InstNoOp(
                            name=f"I-wsplit-{nc.next_id()}",
                            engine=inst.engine,
                            bass_nofuse=True,
                            sync_info=mybir.SyncInfo(on_wait=chunk, on_update=[]),
                        ))
                    si.on_wait = keep
                out.append(inst)
            if len(out) != len(insts):
                try:
                    blk.instructions = out
                except Exception:
                    blk.instructions.clear()
                    blk.instructions.extend(out)


_CACHE = {}


def _get_program(**kw):
    key = tuple(sorted(kw.items()))
    if key not in _CACHE:
        _CACHE[key] = build_program(**kw)
    return _CACHE[key]


def kernel(**inputs):
    nc = _get_program()
    in_maps = prep_inputs(inputs)
    res = run_bass_kernel_spmd(nc, in_maps, core_ids=list(range(NCORES)))
    outs = [res.results[c]["out"] for c in range(NCORES)]   # each [8,11,1000] f32
    full = np.concatenate(outs, axis=0)                     # [64,11,1000]
    return full[:, :, :, None].astype(np.float32)           # [B,T-1,N,1]


if __name__ == "__main__":
    nc = build_program()
    print("program built:", len(nc.m.functions[0].instructions) if hasattr(nc.m.functions[0], "instructions") else "ok")



# revision 2
# speedup vs baseline: 1.0024x; 1.0024x over previous
"""DCRNN decoder (2-layer DCGRU, diffusion graph conv) on 8 trn2 cores, v4.

Sharding: data-parallel over batch B=64 -> 8 batches/core; supports and
weights replicated. No collectives.

v4 on top of v3 (pair packing + cross-step agh0 reuse):
  - Full fp8 (e4m3) DoubleRow matmuls for aggregation AND gate projection:
    2 contraction rows per PE pass. Supports stored as fp8 x256 (avoids
    denormal flush; the x1/256 rescale rides the gate activation's `scale`
    operand for free). Features (h, r*h, x) quantized to fp8; projection
    block-weights fp8 paired over m. Measured end-to-end error vs f32
    reference: ~6.4e-3 (tolerance 2e-2).
  - Aggregations contract kt-pairs; projections contract m-pairs; the x
    selector contracts the two m-half tiles.
  - Elementwise state updates / r*h muls on Pool; PSUM->SBUF copies 2:1
    DVE:ACT.
"""

import sys

import numpy as np
import ml_dtypes

for _p in ("/opt/trn_rl_repo", "/root/.axon_site/_ro/trn_rl_repo"):
    if _p not in sys.path:
        sys.path.append(_p)

import concourse.bass as bass
import concourse.mybir as mybir
import concourse.tile as tile
from concourse.bass import ds
from concourse.bass_utils import run_bass_kernel_spmd

F32 = mybir.dt.float32
BF16 = mybir.dt.bfloat16
F8 = mybir.dt.float8e4

NCORES = 8
BC = 8          # batches per core
N = 1000        # nodes
NPAD = 1024
KT = 8          # node (contraction) tiles of 128
H = 64
M = 4           # supports
NSTEP = 11      # time steps (T-1)
CH = 500        # n_out chunk
NPAIR = 4       # batch pairs
SSCALE = 256.0  # fp8 supports pre-scale

bf16 = ml_dtypes.bfloat16
f8np = ml_dtypes.float8_e4m3

SIG = mybir.ActivationFunctionType.Sigmoid
TANH = mybir.ActivationFunctionType.Tanh
IDENT = mybir.ActivationFunctionType.Identity
COPYF = mybir.ActivationFunctionType.Copy
DR = mybir.MatmulPerfMode.DoubleRow


def build_program(dyn_loop=True, nstep=NSTEP):
    nc = bass.Bass()

    st_d = nc.dram_tensor("st", [128, M, KT, N], F8, kind="ExternalInput")
    xseq_d = nc.dram_tensor("xseq", [nstep, 128, KT, BC], F8, kind="ExternalInput")
    nh0_d = nc.dram_tensor("nh0", [128, KT, NPAIR, 128], F8, kind="ExternalInput")
    nh1_d = nc.dram_tensor("nh1", [128, KT, NPAIR, 128], F8, kind="ExternalInput")
    h0t_d = nc.dram_tensor("h0t", [128, NPAIR, N], F32, kind="ExternalInput")
    h1t_d = nc.dram_tensor("h1t", [128, NPAIR, N], F32, kind="ExternalInput")
    w0blk_d = nc.dram_tensor("w0blk", [128, 3, 2, 2, 128], F8, kind="ExternalInput")
    xsel_d = nc.dram_tensor("xsel", [128, 3, NPAIR, 2, 128], F8, kind="ExternalInput")
    w1blk_d = nc.dram_tensor("w1blk", [128, 6, 2, 2, 128], F8, kind="ExternalInput")
    bias_d = nc.dram_tensor("biases", [128, 6], F32, kind="ExternalInput")
    wproj_d = nc.dram_tensor("wproj", [128, 2], BF16, kind="ExternalInput")
    pbias_d = nc.dram_tensor("pbias", [2, 1], F32, kind="ExternalInput")
    identb_d = nc.dram_tensor("identb", [128, 128], BF16, kind="ExternalInput")
    out_d = nc.dram_tensor("out", [BC, nstep, N], F32, kind="ExternalOutput")

    with tile.TileContext(nc) as tc:
        with (
            tc.tile_pool(name="const", bufs=1) as const,
            tc.tile_pool(name="ag", bufs=6) as ag_pool,
            tc.tile_pool(name="xagp", bufs=4) as xag_pool,
            tc.tile_pool(name="rbuf", bufs=3) as r_pool,
            tc.tile_pool(name="ubuf", bufs=3) as u_pool,
            tc.tile_pool(name="cbuf", bufs=3) as c_pool,
            tc.tile_pool(name="rhbuf", bufs=4) as rh_pool,
            tc.tile_pool(name="htmp", bufs=2) as htmp_pool,
            tc.tile_pool(name="outb", bufs=2) as out_pool,
            tc.tile_pool(name="ps_agg", bufs=2, space="PSUM") as ps_agg,
            tc.tile_pool(name="ps_gate", bufs=2, space="PSUM") as ps_gate,
            tc.tile_pool(name="ps_x", bufs=1, space="PSUM") as ps_x,
            tc.tile_pool(name="ps_tpb", bufs=1, space="PSUM") as ps_tpb,
        ):
            st = const.tile([128, M, KT, N], F8, tag="st")
            nh0 = const.tile([128, KT, NPAIR, 128], F8, tag="nh0")
            nh1 = const.tile([128, KT, NPAIR, 128], F8, tag="nh1")
            nrh = const.tile([128, KT, NPAIR, 128], F8, tag="nrh")
            h0t = const.tile([128, NPAIR, N], F32, tag="h0t")
            h1t = const.tile([128, NPAIR, N], F32, tag="h1t")
            # persistent S_m @ h0 aggregates: [pair, ch, m-pair, m-sub]
            agh0 = const.tile([128, NPAIR, 2, 2, 2, CH], F8, tag="agh0")
            # x natural, padded to 64 free cols (cols 8:64 zero)
            xnat = const.tile([128, KT, 64], F8, tag="xnat")
            w0blk = const.tile([128, 3, 2, 2, 128], F8, tag="w0blk")
            xsel = const.tile([128, 3, NPAIR, 2, 128], F8, tag="xsel")
            w1blk = const.tile([128, 6, 2, 2, 128], F8, tag="w1blk")
            biases = const.tile([128, 6], F32, tag="biases")
            wproj = const.tile([128, 2], BF16, tag="wproj")
            pbias = const.tile([2, 1], F32, tag="pbias")
            identb = const.tile([128, 128], BF16, tag="identb")

            # ---- initial loads ----
            nc.vector.memset(nrh, 0.0)
            nc.vector.memset(xnat, 0.0)
            nc.sync.dma_start(out=st, in_=st_d[:])
            nc.sync.dma_start(out=nh0, in_=nh0_d[:])
            nc.sync.dma_start(out=nh1, in_=nh1_d[:])
            nc.sync.dma_start(out=h0t, in_=h0t_d[:])
            nc.sync.dma_start(out=h1t, in_=h1t_d[:])
            nc.sync.dma_start(out=w0blk, in_=w0blk_d[:])
            nc.sync.dma_start(out=xsel, in_=xsel_d[:])
            nc.sync.dma_start(out=w1blk, in_=w1blk_d[:])
            nc.sync.dma_start(out=biases, in_=bias_d[:])
            nc.sync.dma_start(out=wproj, in_=wproj_d[:])
            nc.sync.dma_start(out=pbias, in_=pbias_d[:])
            nc.sync.dma_start(out=identb, in_=identb_d[:])

            # PSUM->SBUF copies, 2:1 DVE:ACT (Pool cannot read PSUM)
            _eng = [0]

            def copy_rr(out, in_):
                if _eng[0] % 3 != 1:
                    nc.vector.tensor_copy(out, in_)
                else:
                    nc.scalar.activation(out=out, in_=in_, func=COPYF)
                _eng[0] += 1

            def agg_psum_pair(nat, p, ch, mp):
                """DoubleRow aggregation of an m-pair into one 2-bank PSUM
                tile [128, 2, 512] (each half bank-aligned)."""
                aps = ps_agg.tile([128, 2, 512], F32, tag="agg")
                js = ds(ch * CH, CH)
                for sub in range(2):
                    for kp in range(4):
                        nc.tensor.matmul(
                            aps[:, sub, 0:CH],
                            lhsT=nat[:, 2 * kp : 2 * kp + 2, p, :],
                            rhs=st[:, 2 * mp + sub, 2 * kp : 2 * kp + 2, js],
                            start=(kp == 0),
                            stop=(kp == 3),
                            perf_mode=DR,
                            skip_group_check=True,
                        )
                return aps

            def agg_pair_tiles(nat, p, ch):
                """Aggregate vs all 4 supports into 2 m-paired fp8 tiles."""
                out = []
                for mp in range(2):
                    ag = ag_pool.tile([128, 2, CH], F8, tag="ag")
                    aps = agg_psum_pair(nat, p, ch, mp)
                    copy_rr(ag, aps[:, :, 0:CH])
                    out.append(ag)
                return out

            def seed_agh0(p, ch):
                """(Re)compute persistent agh0[:, p, ch, :, :, :] from nh0."""
                for mp in range(2):
                    aps = agg_psum_pair(nh0, p, ch, mp)
                    copy_rr(agh0[:, p, ch, mp, :, :], aps[:, :, 0:CH])

            def xagg(ch):
                """x aggregation, m-stacked at rows 0/64 of 2 halves of one
                fp8 tile [128, 2, CH] (DoubleRow over kt-pairs)."""
                js = ds(ch * CH, CH)
                xs = xag_pool.tile([128, 2, CH], F8, tag="xag")
                for half in range(2):
                    for sub in range(2):
                        m = 2 * half + sub
                        # DoubleRow dst must sit in the base PSUM quadrant,
                        # so each m gets its own [64, CH] tile at base 0
                        xps = ps_x.tile([64, CH], F32, tag="xagg")
                        for kp in range(4):
                            nc.tensor.matmul(
                                xps,
                                lhsT=xnat[:, 2 * kp : 2 * kp + 2, :],
                                rhs=st[:, m, 2 * kp : 2 * kp + 2, js],
                                start=(kp == 0),
                                stop=(kp == 3),
                                perf_mode=DR,
                            )
                        copy_rr(xs[64 * sub : 64 * sub + 64, half, :], xps)
                return xs

            def gate_l0(p, ch, gi, xags, act_fn, out_sb, extra_ags=None):
                """L0 gate (all DoubleRow): m-paired block weights against
                agh0 (or fresh rh aggregates) plus the x selector."""
                js = ds(ch * CH, CH)
                gps = ps_gate.tile([128, CH], F32, tag="gate")
                for mp in range(2):
                    rhs = (extra_ags[mp] if extra_ags is not None
                           else agh0[:, p, ch, mp, :, :])
                    nc.tensor.matmul(
                        gps,
                        lhsT=w0blk[:, gi, mp, :, :],
                        rhs=rhs,
                        start=(mp == 0),
                        stop=False,
                        perf_mode=DR,
                        skip_group_check=True,
                    )
                nc.tensor.matmul(
                    gps,
                    lhsT=xsel[:, gi, p, :, :],
                    rhs=xags,
                    start=False,
                    stop=True,
                    perf_mode=DR,
                    skip_group_check=True,
                )
                nc.scalar.activation(
                    out=out_sb[:, js],
                    in_=gps,
                    func=act_fn,
                    bias=biases[:, gi : gi + 1],
                    scale=1.0 / SSCALE,
                )

            def gate_l1(p, ch, wa, ags_b, wb, act_fn, out_sb, bcol):
                """L1 gate (all DoubleRow): wa against persistent agh0,
                wb against fresh aggregates ags_b."""
                js = ds(ch * CH, CH)
                gps = ps_gate.tile([128, CH], F32, tag="gate")
                for mp in range(2):
                    nc.tensor.matmul(
                        gps,
                        lhsT=w1blk[:, wa, mp, :, :],
                        rhs=agh0[:, p, ch, mp, :, :],
                        start=(mp == 0),
                        stop=False,
                        perf_mode=DR,
                        skip_group_check=True,
                    )
                for mp in range(2):
                    nc.tensor.matmul(
                        gps,
                        lhsT=w1blk[:, wb, mp, :, :],
                        rhs=ags_b[mp],
                        start=False,
                        stop=(mp == 1),
                        perf_mode=DR,
                        skip_group_check=True,
                    )
                nc.scalar.activation(
                    out=out_sb[:, js],
                    in_=gps,
                    func=act_fn,
                    bias=biases[:, bcol : bcol + 1],
                    scale=1.0 / SSCALE,
                )

            def transpose_to_nat(src_ap, dst_nat, p):
                """PE-transpose bf16 src_ap ([128, N], pair-packed) into
                natural fp8 pair tiles dst_nat[:, kt, p, :]."""
                idt = identb
                for half in range(2):
                    tpm = ps_tpb.tile([128, 512], BF16, tag="tpb")
                    for i in range(4):
                        kt = 4 * half + i
                        lo = 128 * kt
                        hi = min(lo + 128, N)
                        w = hi - lo
                        nc.tensor.transpose(
                            tpm[0:w, 128 * i : 128 * i + 128],
                            src_ap[:, lo:hi],
                            idt,
                        )
                    if half == 0:
                        copy_rr(
                            dst_nat[:, 0:4, p, :],
                            tpm.rearrange("p (k f) -> p k f", k=4),
                        )
                    else:
                        copy_rr(
                            dst_nat[:, 4:7, p, :],
                            tpm[:, 0:384].rearrange("p (k f) -> p k f", k=3),
                        )
                        copy_rr(
                            dst_nat[0:104, 7, p, :],
                            tpm[0:104, 384:512],
                        )

            def update_state(ht, p, uT, cT):
                """ht[:, p, :] = cT + uT * (ht[:, p, :] - cT)  (f32).
                Alternate DVE / Pool by pair parity to balance engines."""
                e = nc.vector if p % 2 == 0 else nc.gpsimd
                tmp = htmp_pool.tile([128, N], F32, tag="htmp")
                e.tensor_sub(tmp, ht[:, p, :], cT)
                e.tensor_mul(tmp, uT, tmp)
                e.tensor_add(ht[:, p, :], cT, tmp)

            # ---- prologue: seed agh0 with S_m @ h0_init ----
            for p in range(NPAIR):
                for ch in range(2):
                    seed_agh0(p, ch)

            def step_body(t_iv):
                nc.sync.dma_start(
                    out=xnat[:, :, 0:BC],
                    in_=xseq_d[ds(t_iv, 1), :, :, :].squeeze(0),
                )
                xag = [xagg(ch) for ch in range(2)]

                # ---- layer 0 (no r/u aggregation: reads persistent agh0) ----
                for p in range(NPAIR):
                    rT = r_pool.tile([128, N], BF16, tag="rT")
                    uT = u_pool.tile([128, N], F32, tag="uT")
                    for ch in range(2):
                        gate_l0(p, ch, 0, xag[ch], SIG, rT)
                        gate_l0(p, ch, 1, xag[ch], SIG, uT)
                    rh = rh_pool.tile([128, N], BF16, tag="rh")
                    (nc.gpsimd if p % 2 else nc.vector).tensor_mul(
                        rh, rT, h0t[:, p, :])
                    transpose_to_nat(rh, nrh, p)
                    cT = c_pool.tile([128, N], F32, tag="cT")
                    for ch in range(2):
                        ags = agg_pair_tiles(nrh, p, ch)
                        gate_l0(p, ch, 2, xag[ch], TANH, cT, extra_ags=ags)
                    update_state(h0t, p, uT, cT)
                    h0b = rh_pool.tile([128, N], BF16, tag="rh")
                    (nc.gpsimd if p % 2 == 0 else nc.vector).tensor_copy(
                        h0b, h0t[:, p, :])
                    transpose_to_nat(h0b, nh0, p)

                # ---- layer 1 (rewrites agh0 from the fresh h0) ----
                for p in range(NPAIR):
                    rT = r_pool.tile([128, N], BF16, tag="rT")
                    uT = u_pool.tile([128, N], F32, tag="uT")
                    for ch in range(2):
                        seed_agh0(p, ch)
                        a1 = agg_pair_tiles(nh1, p, ch)
                        gate_l1(p, ch, 0, a1, 1, SIG, rT, 3)
                        gate_l1(p, ch, 2, a1, 3, SIG, uT, 4)
                    rh = rh_pool.tile([128, N], BF16, tag="rh")
                    (nc.gpsimd if p % 2 else nc.vector).tensor_mul(
                        rh, rT, h1t[:, p, :])
                    transpose_to_nat(rh, nrh, p)
                    cT = c_pool.tile([128, N], F32, tag="cT")
                    for ch in range(2):
                        arh = agg_pair_tiles(nrh, p, ch)
                        gate_l1(p, ch, 4, arh, 5, TANH, cT, 5)
                    update_state(h1t, p, uT, cT)
                    # bf16 copy of h1t: transpose source + output projection
                    h1b = rh_pool.tile([128, N], BF16, tag="rh")
                    (nc.gpsimd if p % 2 == 0 else nc.vector).tensor_copy(
                        h1b, h1t[:, p, :])
                    transpose_to_nat(h1b, nh1, p)
                    ob = out_pool.tile([2, N], F32, tag="outb")
                    for ch in range(2):
                        js = ds(ch * CH, CH)
                        ops = ps_gate.tile([2, CH], F32, tag="gate")
                        nc.tensor.matmul(
                            ops,
                            lhsT=wproj[:],
                            rhs=h1b[:, js],
                            start=True,
                            stop=True,
                        )
                        nc.scalar.activation(
                            out=ob[:, js],
                            in_=ops,
                            func=IDENT,
                            bias=pbias[:],
                        )
                    nc.sync.dma_start(
                        out=out_d[2 * p : 2 * p + 2, ds(t_iv, 1), :].squeeze(1),
                        in_=ob,
                    )

            if dyn_loop:
                with tc.For_i(0, nstep, 1, hint_engines=(mybir.EngineType.PE,)) as t:
                    step_body(t)
            else:
                for t in range(nstep):
                    step_body(t)

    _split_excess_waits(nc)
    return nc


def _diag2(w):
    """[64, 64] -> [128, 128] block-diagonal duplicated."""
    blk = np.zeros((128, 128), np.float32)
    blk[0:64, 0:64] = w
    blk[64:128, 64:128] = w
    return blk


def prep_inputs(inputs):
    """Host-side shard + relayout. Returns list of per-core in_maps."""
    S = np.asarray(inputs["supports"], np.float32)           # [M,N,N]
    ih = np.asarray(inputs["init_hidden"], np.float32)       # [2,B,N,H]
    x = np.asarray(inputs["input"], np.float32)[:, :, :, 0]  # [B,T,N]

    # supports, transposed + padded + scaled fp8: st[p,m,kt,j] = S[m][j,128kt+p]
    Sp = np.zeros((M, N, NPAD), np.float32)
    Sp[:, :, :N] = S * SSCALE
    st = Sp.reshape(M, N, KT, 128).transpose(3, 0, 2, 1).astype(f8np).copy()

    # L0 weights: row 0 = x part, rows 1:65 = h part of w0_g[m] ([65, 64])
    w0blk = np.zeros((128, 3, 2, 2, 128), np.float32)
    xsel = np.zeros((128, 3, NPAIR, 2, 128), np.float32)
    for m in range(M):
        mp, sub = divmod(m, 2)
        for gi, wn in enumerate(("w0_r", "w0_u", "w0_c")):
            w = np.asarray(inputs[wn], np.float32)[m]        # [65, 64]
            w0blk[:, gi, mp, sub, :] = _diag2(w[1:65])
            for p in range(NPAIR):
                xsel[64 * sub + 2 * p, gi, p, mp, 0:64] = w[0]
                xsel[64 * sub + 2 * p + 1, gi, p, mp, 64:128] = w[0]

    # L1 weights: rows 0:64 = h0 part, 64:128 = h1 (or r*h1) part
    w1blk = np.zeros((128, 6, 2, 2, 128), np.float32)
    for m in range(M):
        mp, sub = divmod(m, 2)
        for gi, wn in enumerate(("w1_r", "w1_u", "w1_c")):
            w = np.asarray(inputs[wn], np.float32)[m]        # [128, 64]
            w1blk[:, 2 * gi, mp, sub, :] = _diag2(w[0:64])
            w1blk[:, 2 * gi + 1, mp, sub, :] = _diag2(w[64:128])

    biases = np.zeros((128, 6), np.float32)
    for half in (0, 1):
        r0, r1 = half * H, half * H + H
        biases[r0:r1, 0] = inputs["b0_r"]
        biases[r0:r1, 1] = inputs["b0_u"]
        biases[r0:r1, 2] = inputs["b0_c"]
        biases[r0:r1, 3] = inputs["b1_r"]
        biases[r0:r1, 4] = inputs["b1_u"]
        biases[r0:r1, 5] = inputs["b1_c"]
    wproj = np.zeros((128, 2), np.float32)
    wproj[0:H, 0] = np.asarray(inputs["proj_w"], np.float32)[:, 0]
    wproj[H:128, 1] = np.asarray(inputs["proj_w"], np.float32)[:, 0]
    wproj = wproj.astype(bf16)
    pbias = np.full((2, 1), np.asarray(inputs["proj_b"], np.float32).reshape(()),
                    np.float32)
    identb = np.eye(128, dtype=bf16)

    common = dict(st=st, w0blk=w0blk.astype(f8np), xsel=xsel.astype(f8np),
                  w1blk=w1blk.astype(f8np), biases=biases, wproj=wproj,
                  pbias=pbias, identb=identb)

    in_maps = []
    for core in range(NCORES):
        bsl = slice(core * BC, (core + 1) * BC)
        ihc = ih[:, bsl]                                     # [2,8,N,H]
        ihp = np.zeros((2, BC, NPAD, H), np.float32)
        ihp[:, :, :N] = ihc
        # natural pair-packed: [2, 128, KT, NPAIR, 128]
        t = ihp.reshape(2, BC, KT, 128, H).transpose(0, 3, 2, 1, 4)
        nh = t.reshape(2, 128, KT, NPAIR, 2 * H)
        # transposed pair-packed: [2, 128, NPAIR, N]
        htr = ihc.transpose(0, 1, 3, 2).reshape(2, NPAIR, 2 * H, N)
        htr = htr.transpose(0, 2, 1, 3)
        xc = x[bsl, :NSTEP]                                  # [8,11,N]
        xp = np.zeros((BC, NSTEP, NPAD), np.float32)
        xp[:, :, :N] = xc
        xseq = xp.reshape(BC, NSTEP, KT, 128).transpose(1, 3, 2, 0)
        in_maps.append(dict(
            common,
            nh0=nh[0].astype(f8np).copy(),
            nh1=nh[1].astype(f8np).copy(),
            h0t=htr[0].astype(np.float32).copy(),
            h1t=htr[1].astype(np.float32).copy(),
            xseq=xseq.astype(f8np).copy(),
        ))
    return in_maps


_WAIT_CAP = 1


def _split_excess_waits(nc):
    """Walrus codegen here accepts at most 2 sync-wait commands per
    instruction; Tile can emit more.  Move excess waits onto injected
    same-engine no-ops placed immediately before the instruction."""
    for fn in nc.m.functions:
        for blk in fn.blocks:
            insts = list(blk.instructions)
            out = []
            for inst in insts:
                si = getattr(inst, "sync_info", None)
                waits = list(si.on_wait) if si and si.on_wait else []
                if len(waits) > _WAIT_CAP:
                    extra, keep = waits[:-_WAIT_CAP], waits[-_WAIT_CAP:]
                    while extra:
                        chunk, extra = extra[:_WAIT_CAP], extra[_WAIT_CAP:]
                        out.append(mybir.InstNoOp(
                            name=f"I-wsplit-{nc.next_id()}",
                            engine=inst.engine,
                            bass_nofuse=True,
                            sync_info=mybir.SyncInfo(on_wait=chunk, on_update=[]),
                        ))
                    si.on_wait = keep
                out.append(inst)
            if len(out) != len(insts):
                try:
                    blk.instructions = out
                except Exception:
                    blk.instructions.clear()
                    blk.instructions.extend(out)


_CACHE = {}


def _get_program(**kw):
    key = tuple(sorted(kw.items()))
    if key not in _CACHE:
        _CACHE[key] = build_program(**kw)
    return _CACHE[key]


def kernel(**inputs):
    nc = _get_program()
    in_maps = prep_inputs(inputs)
    res = run_bass_kernel_spmd(nc, in_maps, core_ids=list(range(NCORES)))
    outs = [res.results[c]["out"] for c in range(NCORES)]   # each [8,11,1000] f32
    full = np.concatenate(outs, axis=0)                     # [64,11,1000]
    return full[:, :, :, None].astype(np.float32)           # [B,T-1,N,1]


if __name__ == "__main__":
    nc = build_program()
    print("program built ok")
